# revision 7
# baseline (speedup 1.0000x reference)
"""Trainium2 Bass kernel for nn_ModalityMoERouter (expert-choice MoE routing).

Contract: kernel(**inputs) takes the FULL inputs from reference.setup_inputs()
and returns (dispatch, combine), each (16, 8192, 16) float32.

Sharding: data-parallel over batch B=16 across 8 NeuronCores (2 batches/core);
gate weights and expert centers replicated. The global mean(dists) scalar is
computed with one AllReduce (overlapped with the gate MLP).

Math notes:
 - The hard-cap + redistribution step in the reference is exactly a no-op:
   dispatch after the routing floor is <= 0.4*sigmoid + 0.0375 <= 0.4375,
   while cap >= 0.5, so excess == 0 bitwise. It is therefore skipped (t unused).
 - Expert-choice top-k (k=1024 of N=8192 per (batch, expert)) is realized as
   an exact threshold: branch-free bisection on the count of logits above a
   candidate converges to theta in [v_1025, v_1024); mask = logits > theta.

On-chip layout (per core): flat tiles [128, 2048] with partition p = g*16+e
and free f = b*1024 + blk*512 + t, where token n = (blk*8+g)*512 + t.
Per-(b,e) counts are per-partition sums (stt accumulate) summed over the 8
g-groups with a single "same-e" matmul that also replicates the result.
"""

import numpy as np

B = 16
N = 8192
D = 512
H = 256
E = 16
N_CORES = 8
BPC = B // N_CORES
NT = N // 512               # 16 tiles of 512 tokens per batch
KSEL = N * 2 // E           # 1024
ALPHA = min(min(0.05, 0.15 / 4) * E, 1.0)
DSCALE = 1.0 - ALPHA        # 0.4
DFLOOR = ALPHA / E          # 0.0375
N_ITER = 32
W0 = 100.0                  # bisection initial range [-50, 50]
_DEBUG = False

_prog_cache = {}


def _build(debug=False):
    import concourse.bacc as bacc
    import concourse.mybir as mybir
    import concourse.tile as tile

    F32 = mybir.dt.float32
    AO = mybir.AluOpType
    AF = mybir.ActivationFunctionType
    AX = mybir.AxisListType

    nc = bacc.Bacc("TRN2", num_devices=N_CORES)

    tokens_d = nc.dram_tensor("tokens", [BPC, N, D], F32, kind="ExternalInput")
    xyzT_d = nc.dram_tensor("xyzT", [BPC, 3, N], F32, kind="ExternalInput")
    W1_d = nc.dram_tensor("W1", [D + 3, H], F32, kind="ExternalInput")
    W2_d = nc.dram_tensor("W2", [H, E], F32, kind="ExternalInput")
    b1_d = nc.dram_tensor("b1", [H], F32, kind="ExternalInput")
    b2bc_d = nc.dram_tensor("b2bc", [128, 1], F32, kind="ExternalInput")
    crepP_d = nc.dram_tensor("crepP", [128, 3], F32, kind="ExternalInput")
    ident_d = nc.dram_tensor("ident", [128, 128], F32, kind="ExternalInput")
    selg_d = nc.dram_tensor("selg", [8, 128], F32, kind="ExternalInput")
    m2_d = nc.dram_tensor("m2", [128, 128], F32, kind="ExternalInput")

    disp_d = nc.dram_tensor("disp", [BPC, N, E], F32, kind="ExternalOutput")
    comb_d = nc.dram_tensor("comb", [BPC, N, E], F32, kind="ExternalOutput")
    if debug:
        dbg_logits_d = nc.dram_tensor("dbg_logits", [128, 2048], F32,
                                      kind="ExternalOutput")
        dbg_dists_d = nc.dram_tensor("dbg_dists", [128, 2048], F32,
                                     kind="ExternalOutput")

    with tile.TileContext(nc) as tc:
        with tc.tile_pool(name="const", bufs=1) as cpool, \
             tc.tile_pool(name="big", bufs=1) as bigpool, \
             tc.tile_pool(name="work", bufs=2) as work, \
             tc.tile_pool(name="ps", bufs=2, space="PSUM") as ps, \
             tc.tile_pool(name="dram", bufs=1, space="DRAM") as dram:

            # ---- constants ----
            w1_sb = []
            for kc in range(5):
                kch = 128 if kc < 4 else 3
                row = []
                for mc in range(2):
                    t = cpool.tile([kch, 128], F32, tag=f"w1_{kc}_{mc}",
                                   name=f"w1_{kc}_{mc}")
                    nc.sync.dma_start(
                        out=t[:], in_=W1_d[kc * 128:kc * 128 + kch,
                                           mc * 128:(mc + 1) * 128])
                    row.append(t)
                w1_sb.append(row)
            w2_sb = []
            for c in range(2):
                t = cpool.tile([128, E], F32, tag=f"w2_{c}", name=f"w2_{c}")
                nc.sync.dma_start(out=t[:], in_=W2_d[c * 128:(c + 1) * 128, :])
                w2_sb.append(t)
            b1_sb = []
            for mc in range(2):
                t = cpool.tile([128, 1], F32, tag=f"b1_{mc}", name=f"b1_{mc}")
                nc.sync.dma_start(out=t[:],
                                  in_=b1_d[mc * 128:(mc + 1) * 128].unsqueeze(1))
                b1_sb.append(t)
            b2bc_sb = cpool.tile([128, 1], F32, tag="b2bc", name="b2bc")
            nc.sync.dma_start(out=b2bc_sb[:], in_=b2bc_d[:])
            crepP_sb = cpool.tile([128, 3], F32, tag="crepP", name="crepP")
            nc.sync.dma_start(out=crepP_sb[:], in_=crepP_d[:])
            ident_sb = cpool.tile([128, 128], F32, tag="ident", name="ident")
            nc.sync.dma_start(out=ident_sb[:], in_=ident_d[:])
            selg_sb = cpool.tile([8, 128], F32, tag="selg", name="selg")
            nc.sync.dma_start(out=selg_sb[:], in_=selg_d[:])
            m2_sb = cpool.tile([128, 128], F32, tag="m2", name="m2")
            nc.sync.dma_start(out=m2_sb[:], in_=m2_d[:])
            ones_1x128 = cpool.tile([1, 128], F32, tag="o1x", name="o1x")
            nc.vector.memset(ones_1x128[:], 1.0)
            ones_128x1 = cpool.tile([128, 1], F32, tag="ox1", name="ox1")
            nc.vector.memset(ones_128x1[:], 1.0)
            ones_wide = cpool.tile([128, 1024], F32, tag="onesw", name="onesw")
            nc.vector.memset(ones_wide[:], 1.0)

            # ---- persistent tiles ----
            logits_A = bigpool.tile([128, 2048], F32, tag="logits", name="logits")
            dists_A = bigpool.tile([128, 2048], F32, tag="dists", name="dists")
            sig_A = bigpool.tile([128, 2048], F32, tag="sig", name="sig")

            # ============ Phase A: distances + global mean =================
            for b in range(BPC):
                for blk in range(2):
                    off = b * 1024 + blk * 512
                    acc = work.tile([128, 512], F32, tag="dacc", name="dacc")
                    for c in range(3):
                        xrow = work.tile([8, 512], F32, tag="xrow", name="xrow")
                        nc.sync.dma_start(
                            out=xrow[:],
                            in_=xyzT_d[b, c, blk * 4096:(blk + 1) * 4096]
                                .rearrange("(g t) -> g t", g=8))
                        p_xt = ps.tile([128, 512], F32, tag="xt", name="p_xt")
                        nc.tensor.matmul(p_xt[:], selg_sb[:], xrow[:],
                                         start=True, stop=True)
                        d = work.tile([128, 512], F32, tag="dtmp", name="dtmp")
                        nc.vector.tensor_scalar(out=d[:], in0=p_xt[:],
                                                scalar1=crepP_sb[:, c:c + 1],
                                                scalar2=None, op0=AO.subtract)
                        if c == 0:
                            nc.vector.tensor_tensor(out=acc[:], in0=d[:], in1=d[:],
                                                    op=AO.mult)
                        else:
                            nc.vector.tensor_tensor(out=d[:], in0=d[:], in1=d[:],
                                                    op=AO.mult)
                            nc.vector.tensor_tensor(out=acc[:], in0=acc[:], in1=d[:],
                                                    op=AO.add)
                    # sqrt + one Heron step (ACT Sqrt LUT alone is ~2e-5 rel)
                    y0 = work.tile([128, 512], F32, tag="y0", name="y0")
                    nc.scalar.activation(y0[:], acc[:], AF.Sqrt)
                    r = work.tile([128, 512], F32, tag="ry", name="ry")
                    nc.vector.reciprocal(r[:], y0[:])
                    nc.vector.tensor_tensor(out=r[:], in0=acc[:], in1=r[:], op=AO.mult)
                    nc.vector.tensor_tensor(out=r[:], in0=r[:], in1=y0[:], op=AO.add)
                    nc.vector.tensor_scalar(out=dists_A[:, off:off + 512], in0=r[:],
                                            scalar1=0.5, scalar2=None, op0=AO.mult)

            rsum = work.tile([128, 1], F32, tag="rsum", name="rsum")
            nc.vector.tensor_reduce(out=rsum[:], in_=dists_A[:], axis=AX.X, op=AO.add)
            p_tot = ps.tile([1, 1], F32, tag="xt", name="p_tot")
            nc.tensor.matmul(p_tot[:], ones_128x1[:], rsum[:], start=True, stop=True)
            s_tot = work.tile([1, 1], F32, tag="stot", name="stot")
            nc.vector.tensor_copy(s_tot[:], p_tot[:])
            p_bc = ps.tile([128, 1], F32, tag="xt", name="p_bc")
            nc.tensor.matmul(p_bc[:], ones_1x128[:], s_tot[:], start=True, stop=True)
            sb_bc = work.tile([128, 1], F32, tag="sbbc", name="sbbc")
            nc.vector.tensor_copy(sb_bc[:], p_bc[:])
            cc_in = dram.tile([128, 1], F32)
            cc_out = dram.tile([128, 1], F32, addr_space="Shared")
            nc.sync.dma_start(out=cc_in[:], in_=sb_bc[:])
            nc.gpsimd.collective_compute(
                "AllReduce", AO.add, ins=[cc_in.opt()], outs=[cc_out.opt()],
                replica_groups=[list(range(N_CORES))])
            S_sb = bigpool.tile([128, 1], F32, tag="S", name="S")
            nc.sync.dma_start(out=S_sb[:], in_=cc_out[:])
            m_sb = bigpool.tile([128, 1], F32, tag="m", name="m")
            nc.vector.tensor_scalar(out=m_sb[:], in0=S_sb[:],
                                    scalar1=1.0 / (B * N * E), scalar2=1e-6,
                                    op0=AO.mult, op1=AO.add)
            r_sb = bigpool.tile([128, 1], F32, tag="r", name="r")
            nc.vector.reciprocal(r_sb[:], m_sb[:])
            a_sb = bigpool.tile([128, 1], F32, tag="a", name="a")
            nc.vector.tensor_scalar(out=a_sb[:], in0=r_sb[:], scalar1=-1.0,
                                    scalar2=None, op0=AO.mult)

            # ---- bisect state ----
            lo, mask = [], []
            for b in range(BPC):
                lo.append(bigpool.tile([128, 1], F32, tag=f"lo{b}", name=f"lo{b}"))
                mask.append(bigpool.tile([128, 1024], F32, tag=f"mask{b}",
                                         name=f"mask{b}"))
                nc.vector.memset(lo[b][:], -W0 / 2)

            def mlp_batch(b):
                for T in range(NT):
                    blk, g = T // 8, T % 8
                    nat = []
                    for s in range(4):
                        t = work.tile([128, 512], F32, tag="nat", name="nat", bufs=6)
                        r0 = 512 * T + 128 * s
                        nc.sync.dma_start(out=t[:], in_=tokens_d[b, r0:r0 + 128, :])
                        nat.append(t)
                    tokT = []
                    for c in range(4):
                        p_t = ps.tile([128, 512], F32, tag="tokT", name="p_tokT")
                        for s in range(4):
                            nc.tensor.transpose(p_t[:, s * 128:(s + 1) * 128],
                                                nat[s][:, c * 128:(c + 1) * 128],
                                                ident_sb[:])
                        t_sb = work.tile([128, 512], F32, tag=f"tokT{c}",
                                         name=f"tokT{c}")
                        if c % 2 == 0:
                            nc.vector.tensor_copy(t_sb[:], p_t[:])
                        else:
                            nc.scalar.activation(t_sb[:], p_t[:], AF.Copy)
                        tokT.append(t_sb)
                    xyzw = work.tile([3, 512], F32, tag="xyzw", name="xyzw")
                    nc.sync.dma_start(out=xyzw[:],
                                      in_=xyzT_d[b, :, 512 * T:512 * (T + 1)])
                    h_sb = []
                    for mc in range(2):
                        p_h = ps.tile([128, 512], F32, tag="h", name="p_h")
                        for kc in range(5):
                            rhs = tokT[kc][:] if kc < 4 else xyzw[:]
                            nc.tensor.matmul(p_h[:], w1_sb[kc][mc][:], rhs,
                                             start=(kc == 0), stop=(kc == 4))
                        t_h = work.tile([128, 512], F32, tag=f"h{mc}", name=f"h{mc}")
                        nc.scalar.activation(t_h[:], p_h[:], AF.Gelu,
                                             bias=b1_sb[mc][:], scale=1.0)
                        h_sb.append(t_h)
                    p_l2 = ps.tile([16, 512], F32, tag="l2", name="p_l2")
                    for c in range(2):
                        nc.tensor.matmul(p_l2[:], w2_sb[c][:], h_sb[c][:],
                                         start=(c == 0), stop=(c == 1))
                    t_st = work.tile([16, 512], F32, tag="l2st", name="l2st", bufs=3)
                    nc.scalar.activation(t_st[:], p_l2[:], AF.Copy)
                    off = b * 1024 + blk * 512
                    nc.sync.dma_start(
                        out=logits_A[16 * g:16 * (g + 1), off:off + 512],
                        in_=t_st[:])

            def finalize_logits(b):
                sl = slice(b * 1024, (b + 1) * 1024)
                # logits = content + b2 + a*dists
                nc.vector.scalar_tensor_tensor(
                    out=logits_A[:, sl], in0=dists_A[:, sl], scalar=a_sb[:],
                    in1=logits_A[:, sl], op0=AO.mult, op1=AO.add)
                nc.vector.tensor_scalar(out=logits_A[:, sl], in0=logits_A[:, sl],
                                        scalar1=b2bc_sb[:], scalar2=None, op0=AO.add)
                nc.scalar.activation(sig_A[:, sl], logits_A[:, sl], AF.Sigmoid)

            def bisect(b):
                sl = slice(b * 1024, (b + 1) * 1024)
                t_mid = work.tile([128, 1], F32, tag=f"mid{b}", name=f"mid{b}",
                                  bufs=3)
                t_acc = work.tile([128, 1], F32, tag=f"pacc{b}", name=f"pacc{b}",
                                  bufs=3)
                t_s = work.tile([128, 1], F32, tag=f"sel{b}", name=f"sel{b}", bufs=3)
                for i in range(N_ITER):
                    w = W0 / (2 ** (i + 1))
                    nc.vector.tensor_scalar(out=t_mid[:], in0=lo[b][:], scalar1=w,
                                            scalar2=None, op0=AO.add)
                    nc.vector.scalar_tensor_tensor(
                        out=mask[b][:], in0=logits_A[:, sl], scalar=t_mid[:],
                        in1=ones_wide[:], op0=AO.is_gt, op1=AO.mult,
                        accum_out=t_acc[:])
                    p_cnt = ps.tile([128, 1], F32, tag="xt", name="p_cnt")
                    nc.tensor.matmul(p_cnt[:], m2_sb[:], t_acc[:],
                                     start=True, stop=True)
                    nc.vector.tensor_scalar(out=t_s[:], in0=p_cnt[:],
                                            scalar1=float(KSEL), scalar2=None,
                                            op0=AO.is_ge)
                    nc.vector.scalar_tensor_tensor(
                        out=lo[b][:], in0=t_s[:], scalar=w, in1=lo[b][:],
                        op0=AO.mult, op1=AO.add)

            # phase order: MLP b0 -> finalize/bisect b0 (overlaps MLP b1) -> ...
            mlp_batch(0)
            finalize_logits(0)
            bisect(0)
            mlp_batch(1)
            finalize_logits(1)
            bisect(1)

            if debug:
                nc.sync.dma_start(out=dbg_dists_d[:], in_=dists_A[:])
                nc.sync.dma_start(out=dbg_logits_d[:], in_=logits_A[:])

            # ============ Phase E: dispatch/combine + outputs ==============
            for b in range(BPC):
                sl = slice(b * 1024, (b + 1) * 1024)
                nc.vector.scalar_tensor_tensor(
                    out=logits_A[:, sl], in0=logits_A[:, sl], scalar=lo[b][:],
                    in1=sig_A[:, sl], op0=AO.is_gt, op1=AO.mult)
            nc.vector.tensor_scalar(out=logits_A[:], in0=logits_A[:],
                                    scalar1=DSCALE, scalar2=DFLOOR,
                                    op0=AO.mult, op1=AO.add)
            for b in range(BPC):
                out_view_d = disp_d[b].rearrange(
                    "(blk g q t) e -> blk q t g e", blk=2, g=8, q=4)
                out_view_c = comb_d[b].rearrange(
                    "(blk g q t) e -> blk q t g e", blk=2, g=8, q=4)
                for blk in range(2):
                    for q in range(4):
                        off = b * 1024 + blk * 512 + q * 128
                        p_o = ps.tile([128, 128], F32, tag="h", name="p_o")
                        nc.tensor.transpose(p_o[:], logits_A[:, off:off + 128],
                                            ident_sb[:])
                        t_o = work.tile([128, 128], F32, tag="outT", name="outT",
                                        bufs=3)
                        nc.vector.tensor_copy(t_o[:], p_o[:])
                        t_sden = work.tile([128, 8], F32, tag="sden", name="sden",
                                           bufs=3)
                        nc.vector.tensor_reduce(
                            out=t_sden[:],
                            in_=t_o[:].rearrange("t (g e) -> t g e", g=8),
                            axis=AX.X, op=AO.add)
                        nc.vector.tensor_scalar(out=t_sden[:], in0=t_sden[:],
                                                scalar1=1e-8, scalar2=None,
                                                op0=AO.add)
                        t_rden = work.tile([128, 8], F32, tag="rden", name="rden",
                                           bufs=3)
                        nc.vector.reciprocal(t_rden[:], t_sden[:])
                        t_c = work.tile([128, 128], F32, tag="outC", name="outC",
                                        bufs=3)
                        nc.vector.tensor_tensor(
                            out=t_c[:].rearrange("t (g e) -> t g e", g=8),
                            in0=t_o[:].rearrange("t (g e) -> t g e", g=8),
                            in1=t_rden[:].unsqueeze(2).broadcast_to([128, 8, E]),
                            op=AO.mult)
                        nc.sync.dma_start(
                            out=out_view_d[blk, q],
                            in_=t_o[:].rearrange("t (g e) -> t g e", g=8))
                        nc.sync.dma_start(
                            out=out_view_c[blk, q],
                            in_=t_c[:].rearrange("t (g e) -> t g e", g=8))

    nc.finalize()
    return nc


def _get_prog(debug=False):
    key = ("prog", debug)
    if key not in _prog_cache:
        _prog_cache[key] = _build(debug)
    return _prog_cache[key]


def make_in_maps(inputs):
    tokens = np.ascontiguousarray(np.asarray(inputs["tokens"], dtype=np.float32))
    xyz = np.ascontiguousarray(np.asarray(inputs["spatial_xyz"], dtype=np.float32))
    W1 = np.ascontiguousarray(np.asarray(inputs["W1"], dtype=np.float32))
    b1 = np.asarray(inputs["b1"], dtype=np.float32)
    W2 = np.ascontiguousarray(np.asarray(inputs["W2"], dtype=np.float32))
    b2 = np.asarray(inputs["b2"], dtype=np.float32)
    centers = np.asarray(inputs["centers"], dtype=np.float32)

    b2bc = np.ascontiguousarray(np.tile(b2, 8)[:, None].astype(np.float32))
    crepP = np.ascontiguousarray(np.tile(centers, (8, 1)).astype(np.float32))
    ident = np.eye(128, dtype=np.float32)
    selg = np.ascontiguousarray(np.repeat(np.eye(8, dtype=np.float32), 16, axis=1))
    m2 = np.ascontiguousarray(
        (np.arange(128)[:, None] % 16 == np.arange(128)[None, :] % 16)
        .astype(np.float32))

    in_maps = []
    for c in range(N_CORES):
        sl = slice(BPC * c, BPC * (c + 1))
        in_maps.append({
            "tokens": tokens[sl],
            "xyzT": np.ascontiguousarray(xyz[sl].transpose(0, 2, 1)),
            "W1": W1, "W2": W2, "b1": b1,
            "b2bc": b2bc, "crepP": crepP, "ident": ident,
            "selg": selg, "m2": m2,
        })
    return in_maps


def kernel(**inputs):
    from concourse.bass_utils import run_bass_kernel_spmd

    nc = _get_prog(_DEBUG)
    in_maps = make_in_maps(inputs)
    res = run_bass_kernel_spmd(nc, in_maps, list(range(N_CORES)))
    dispatch = np.concatenate([res.results[c]["disp"] for c in range(N_CORES)], axis=0)
    combine = np.concatenate([res.results[c]["comb"] for c in range(N_CORES)], axis=0)
    if _DEBUG:
        kernel._dbg = [(res.results[c]["dbg_logits"], res.results[c]["dbg_dists"])
                       for c in range(N_CORES)]
    return dispatch, combine
